# revision 68
# baseline (speedup 1.0000x reference)
"""Trainium2 Bass kernel for AffinityMatrixConstructLayer.

Math: M[(i2,i1),(k2,k1)] = sum_{j2,j1} G2[i2,j2]H2[k2,j2] Me[j2,j1]
                            G1[i1,j1]H1[k1,j1]  + diag(Mp)
where Me rows play the j2 role (e1==e2==192 makes the kron index
arithmetic alias me's ef1-row index to j2).

Structure exploited per core c (owns i2 block-rows [6c, 6c+6)):
  - only graph-2 edges with head in range contribute; host permutes
    edges + ef1 rows so the owned slice is compact (C=32 = data max)
  - the output is ~0.4% dense: only graph-1 edge columns + the
    diagonal are nonzero, so the device computes a compact
    [288, NC=256] result (cols = diag48 | edge cols) and the host
    scatters it into the zero matrix
  - bf16 W stream (4MB/core) via SWDGE (nc.gpsimd, 16-engine spray,
    8x512KB tiles all in flight); Wn first so its matvec gather+tanh
    overlaps the We half of the stream
  - coeff = tanh(W@gw+b): PE matvec with gw stationary (1-col
    LDWEIGHTS, host-transposed W), [1,512] psum rows gathered into
    [128,16] via 16 PE transposes; tanh via exp (one ACT table set)
  - one merged affinity GEMM [ef1_own | x1_own] x [x2 | ef2]
  - P = Me_own @ B1c (compact one-hot pattern host-built, bf16)
  - diag(Mp) folded into the final GEMM: p_sb rows C..C+5 carry mp
    rows at compact cols 0:48, cv rows C..C+5 make s2 route them to
    the k2rot=0 output rows. No DRAM bounce, no separate diag add.
  - PE pre-warmed with junk matmuls on the first W tile.
"""

import sys

for _p in ("/opt/trn_rl_repo", "/root/.axon_site/_ro/trn_rl_repo"):
    if _p not in sys.path:
        sys.path.insert(0, _p)

import numpy as np
import ml_dtypes

import concourse.bass as bass
import concourse.mybir as mybir
from concourse.tile import TileContext
from concourse.masks import make_identity
from concourse.bass_utils import run_bass_kernel_spmd

F32 = mybir.dt.float32
BF16 = mybir.dt.bfloat16
AF = mybir.ActivationFunctionType
ALU = mybir.AluOpType

N_CORES = 8
N = 48          # nodes per graph
E = 192         # edges per graph
D = 1024        # feature dim
I2P = N // N_CORES          # 6 block-rows per core
ROWS = I2P * N              # 288 output rows per core
COLS = N * N                # 2304
C = 32                      # padded owned-edge capacity per core
CD = C + 6                  # + 6 mp-diagonal delta rows
LW = C + 6                  # merged affinity lhs width (ef1_own | x1_own)
RW = N + E                  # merged affinity rhs width (x2 | ef2) = 240
KC = D // 128               # 8 contraction chunks
NC = 256                    # padded compact output columns (diag48 | edge cols)

_CACHE: dict = {}
LAST_RESULTS = None


def _split_multiwaits(nc):
    """This walrus build encodes at most one sync-wait per instruction.
    Move extra waits onto injected single-wait drains on the same engine
    (engine queues execute in order, so semantics are preserved)."""
    for f in nc.m.functions:
        for blk in f.blocks:
            out = []
            for inst in blk.instructions:
                si = getattr(inst, "sync_info", None)
                if si is not None and si.on_wait and len(si.on_wait) > 1:
                    waits = list(si.on_wait)
                    for w in waits[:-1]:
                        d = mybir.InstDrain(
                            name=nc.get_next_instruction_name(),
                            ins=[], outs=[], bass_is_fusable=False)
                        d.engine = inst.engine
                        d.sync_info = mybir.SyncInfo(on_wait=[w], on_update=[])
                        out.append(d)
                    si.on_wait = waits[-1:]
                out.append(inst)
            try:
                blk.instructions[:] = out
            except TypeError:
                blk.instructions = out


def _build() -> bass.Bass:
    if "nc" in _CACHE:
        return _CACHE["nc"]
    nc = bass.Bass(trn_type="TRN2", num_devices=N_CORES)

    # Wn/We are host-TRANSPOSED+tiled: [128, 4*2048]; column-chunk g holds
    # din rows [256g, 256g+256) as two 1024-wide k-slices, so the PE
    # matvec can use gw as the stationary operand (1-col LDWEIGHTS)
    d_Wn = nc.dram_tensor("Wn", [128, KC * D], BF16,
                          kind="ExternalInput")
    d_We = nc.dram_tensor("We", [128, KC * D], BF16,
                          kind="ExternalInput")
    d_gw = nc.dram_tensor("gw", [128, KC], BF16, kind="ExternalInput")
    d_bnbe = nc.dram_tensor("bnbe", [128, 16], F32, kind="ExternalInput")
    d_lhs = nc.dram_tensor("lhs", [128, KC * LW], BF16, kind="ExternalInput")
    d_rhs = nc.dram_tensor("rhs", [128, KC * RW], BF16, kind="ExternalInput")
    d_b1 = nc.dram_tensor("b1", [E, NC], BF16, kind="ExternalInput")
    d_cv = nc.dram_tensor("cv", [CD, 6], F32, kind="ExternalInput")
    d_out = nc.dram_tensor("out", [ROWS, NC], F32, kind="ExternalOutput")

    with TileContext(nc) as tc:
        with (
            tc.tile_pool(name="const", bufs=1) as cpool,
            tc.tile_pool(name="wstream", bufs=8) as wpool,
            tc.tile_pool(name="scratch", bufs=2) as spool,
            tc.tile_pool(name="orow", bufs=3) as opool,
            tc.tile_pool(name="paff", bufs=1, space="PSUM") as paff,
            tc.tile_pool(name="pmv", bufs=1, space="PSUM") as pmv,
            tc.tile_pool(name="pout", bufs=2, space="PSUM") as pout,
            tc.tile_pool(name="pfin", bufs=2, space="PSUM") as pfin,
        ):
            # ---- bulk stream on SWDGE (gpsimd): 16-engine spray, 512KB
            # chunks. Emitted FIRST so Q7 descriptor-gen precedes its
            # other (iota/memset) work.
            wn_tiles, we_tiles = [], []
            for g in range(4):
                wn = wpool.tile([128, 2 * D], BF16, tag="w", name=f"wn{g}")
                nc.gpsimd.dma_start(
                    out=wn, in_=d_Wn[:, 2 * D * g:2 * D * (g + 1)])
                wn_tiles.append(wn)
            for g in range(2):
                we = wpool.tile([128, 4 * D], BF16, tag="we", name=f"we{g}")
                nc.gpsimd.dma_start(
                    out=we, in_=d_We[:, 4 * D * g:4 * D * (g + 1)])
                we_tiles.append(we)
            # ---- everything else on the two HWDGE queues (idle during
            # the SWDGE W stream): sync = bb, rhs, lhs, b1_hi;
            # scalar = cv, gw, b1_lo
            bb_t = cpool.tile([128, 16], F32, tag="bb", name="bb")
            nc.sync.dma_start(out=bb_t, in_=d_bnbe[:, :])
            rhs = cpool.tile([128, KC * RW], BF16, tag="rhs", name="rhs")
            nc.sync.dma_start(out=rhs, in_=d_rhs[:, :])
            lhs = cpool.tile([128, KC * LW], BF16, tag="lhs", name="lhs")
            nc.sync.dma_start(out=lhs, in_=d_lhs[:, :])
            b1_hi = cpool.tile([128, NC], BF16, tag="b1_hi", name="b1_hi")
            nc.sync.dma_start(out=b1_hi, in_=d_b1[0:128, :])
            cv = cpool.tile([CD, 6], F32, tag="cv", name="cv")
            nc.scalar.dma_start(out=cv, in_=d_cv[:, :])
            gwp = cpool.tile([128, KC], BF16, tag="gwp", name="gwp")
            nc.scalar.dma_start(out=gwp, in_=d_gw[:, :])
            b1_lo = cpool.tile([64, NC], BF16, tag="b1_lo", name="b1_lo")
            nc.scalar.dma_start(out=b1_lo, in_=d_b1[128:192, :])

            # ---------- constants (built during stream) -----------------
            ident = cpool.tile([128, 128], F32, tag="ident", name="ident")
            make_identity(nc, ident)
            iota96 = cpool.tile([CD, 96], F32, tag="iota96", name="i96")
            nc.gpsimd.iota(iota96, pattern=[[1, 96]], base=0,
                           channel_multiplier=0,
                           allow_small_or_imprecise_dtypes=True)

            # ACT table preload overlaps the W stream
            dum = spool.tile([1, 1], F32, tag="dum", name="dum")
            nc.vector.memset(dum, 0.0)
            nc.scalar.activation(dum, dum, AF.Exp)

            # p_sb rows C..C+5 (mp diag deltas): zero the background once
            # (tile padded to 64 partitions for gpsimd's 32-alignment rule)
            p_sb = cpool.tile([64, NC], BF16, tag="p_sb", name="p_sb")
            nc.gpsimd.memset(p_sb, 0.0)

            # s2 per pair: col (64*(i2%2) + k2rot) hot iff cv matches;
            # rows C..C+5 route the mp-diag delta rows of p_sb
            s2p = []
            for pa in range(3):
                sa = spool.tile([CD, 96], F32, tag="s2a", name=f"s2a{pa}")
                nc.vector.tensor_tensor(
                    sa, iota96,
                    cv[:, 2 * pa:2 * pa + 1].broadcast_to((CD, 96)),
                    ALU.is_equal)
                sb = spool.tile([CD, 96], F32, tag="s2b", name=f"s2b{pa}")
                nc.vector.tensor_tensor(
                    sb, iota96,
                    cv[:, 2 * pa + 1:2 * pa + 2].broadcast_to((CD, 96)),
                    ALU.is_equal)
                st = cpool.tile([CD, 96], BF16, tag=f"s2{pa}", name=f"s2{pa}")
                nc.vector.tensor_tensor(st, sa, sb, ALU.add)
                s2p.append(st)

            # ---------- streaming PE matvec: gw stationary (1-col weights)
            # Wn fully streamed first, so its gather+tanh overlaps the We
            # stream. Junk matmuls on the first tile warm the HAM early.
            ones33 = cpool.tile([33, 1], F32, tag="ones33", name="ones33")
            nc.vector.memset(ones33, 1.0)
            coeff = cpool.tile([128, 16], F32, tag="coeff", name="coeff")
            pmvt = pmv.tile([128, 16], F32, tag="pmvt", name="pmvt")

            warm = pfin.tile([128, 512], F32, tag="pf", name="wrm")
            for j in range(6):
                nc.tensor.matmul(warm, wn_tiles[0][:, 0:128],
                                 wn_tiles[0][:, 0:512],
                                 start=True, stop=True)

            def matvec_half(m, tiles, pm):
                ntile = len(tiles)
                spt = KC // ntile  # k-slices per tile
                for g in range(ntile):
                    for s in range(spt):
                        k = spt * g + s
                        for h in range(2):
                            nc.tensor.matmul(
                                pm[32 * h:32 * h + 1, :], gwp[:, k:k + 1],
                                tiles[g][:, 1024 * s + 512 * h:
                                         1024 * s + 512 * h + 512],
                                start=(k == 0), stop=(k == KC - 1))
                # gather [1,512] psum rows -> pmvt cols via PE transposes
                mvs = spool.tile([33, 512], F32, tag=f"mvs{m}", name=f"mvs{m}")
                nc.vector.tensor_copy(mvs[:, 0:256], pm[:, 0:256])
                nc.scalar.copy(mvs[:, 256:512], pm[:, 256:512])
                for h in range(2):
                    for kc in range(4):
                        col = m * 8 + 4 * h + kc
                        nc.tensor.transpose(
                            pmvt[:, col:col + 1],
                            mvs[32 * h:32 * h + 1, 128 * kc:128 * kc + 128],
                            ones33[32 * h:32 * h + 1, 0:1])
                # tanh(mv + b) via exp on this half's 8 columns
                cs = slice(m * 8, m * 8 + 8)
                mv2 = spool.tile([128, 8], F32, tag=f"mv2{m}", name=f"mv2{m}")
                nc.vector.scalar_tensor_tensor(
                    out=mv2, in0=pmvt[:, cs], scalar=2.0, op0=ALU.mult,
                    in1=bb_t[:, cs], op1=ALU.add)
                et = spool.tile([128, 8], F32, tag=f"et{m}", name=f"et{m}")
                nc.scalar.activation(et, mv2, AF.Exp)
                nc.vector.tensor_scalar_add(et, et, 1.0)
                rt = spool.tile([128, 8], F32, tag=f"rt{m}", name=f"rt{m}")
                nc.vector.reciprocal(rt, et)
                nc.vector.tensor_scalar(coeff[:, cs], rt, -2.0, 1.0,
                                        ALU.mult, ALU.add)

            pmva = pmv.tile([33, 512], F32, tag="pmva", name="pmva")
            matvec_half(0, wn_tiles, pmva)
            pmvb = pmv.tile([33, 512], F32, tag="pmvb", name="pmvb")
            matvec_half(1, we_tiles, pmvb)

            # ---------- scaled lhs + merged affinity GEMM ---------------
            al = cpool.tile([128, KC * LW], BF16, tag="al", name="al")
            al3 = al.rearrange("p (k n) -> p k n", n=LW)
            lhs3 = lhs.rearrange("p (k n) -> p k n", n=LW)
            nc.vector.tensor_tensor(
                al3[:, :, 0:C], lhs3[:, :, 0:C],
                coeff[:, KC:16].unsqueeze(2).broadcast_to((128, KC, C)),
                ALU.mult)
            nc.vector.tensor_tensor(
                al3[:, :, C:LW], lhs3[:, :, C:LW],
                coeff[:, 0:KC].unsqueeze(2).broadcast_to((128, KC, I2P)),
                ALU.mult)
            aff = paff.tile([LW, RW], F32, tag="aff", name="aff")
            rhs3 = rhs.rearrange("p (k n) -> p k n", n=RW)
            for k in range(KC):
                nc.tensor.matmul(aff, al3[:, k, :], rhs3[:, k, :],
                                 start=(k == 0), stop=(k == KC - 1))


            # ---------- softplus_relu: relu(relu(x) + ln(e^-.5 + e^(-|x|-.5)))
            nh05 = cpool.tile([LW, 1], F32, tag="nh05", name="nh05")
            nc.vector.memset(nh05, -0.5)
            lnc = cpool.tile([LW, 1], F32, tag="lnc", name="lnc")
            nc.vector.memset(lnc, 0.6065306597126334)
            ab = spool.tile([LW, RW], F32, tag="sp_ab", name="sp_ab")
            nc.scalar.activation(ab, aff, AF.Abs)
            ex = spool.tile([LW, RW], F32, tag="sp_ex", name="sp_ex")
            nc.scalar.activation(ex, ab, AF.Exp, scale=-1.0, bias=nh05)
            ln = spool.tile([LW, RW], F32, tag="sp_ln", name="sp_ln")
            nc.scalar.activation(ln, ex, AF.Ln, bias=lnc)
            pre = spool.tile([LW, RW], F32, tag="sp_pre", name="sp_pre")
            nc.vector.scalar_tensor_tensor(
                out=pre, in0=aff, scalar=0.0, op0=ALU.max,
                in1=ln, op1=ALU.add)
            # (final relu is folded into the consumers below)

            # mp diag deltas: compact cols 0..47 ARE the diagonal, so this
            # is a plain relu-copy into p_sb delta rows (partitions C..C+5)
            nc.vector.scalar_tensor_tensor(
                out=p_sb[C:CD, 0:N], in0=pre[C:CD, 0:N], scalar=0.0,
                op0=ALU.max, in1=pre[C:CD, 0:N], op1=ALU.bypass)

            # MeT via PE transposes (relu folded into the psum->sbuf copy)
            ptm1 = pout.tile([128, C], F32, tag="po", name="ptm1")
            nc.tensor.transpose(ptm1, pre[0:C, N:N + 128], ident[0:C, 0:C])
            met_hi = cpool.tile([128, C], BF16, tag="met_hi", name="met_hi")
            nc.scalar.activation(met_hi, ptm1, AF.Relu)
            ptm2 = pout.tile([64, C], F32, tag="po", name="ptm2")
            nc.tensor.transpose(ptm2, pre[0:C, N + 128:N + 192],
                                ident[0:C, 0:C])
            met_lo = cpool.tile([64, C], BF16, tag="met_lo", name="met_lo")
            nc.scalar.activation(met_lo, ptm2, AF.Relu)

            # ---------- P = Me_own @ B1c  [C, 256] -> bf16 p_sb ---------
            pp = pout.tile([C, NC], F32, tag="po", name="pp")
            nc.tensor.matmul(pp, met_hi, b1_hi, start=True, stop=False)
            nc.tensor.matmul(pp, met_lo, b1_lo, start=False, stop=True)
            nc.vector.tensor_copy(p_sb[0:C, 0:NC // 2], pp[:, 0:NC // 2])
            nc.scalar.copy(p_sb[0:C, NC // 2:], pp[:, NC // 2:])

            # ---------- finals: orow = s2^T @ p_sb per pair + out DMA ---
            for pa in range(3):
                i2a, i2b = 2 * pa, 2 * pa + 1
                orow = opool.tile([96, NC], F32, tag="orow", name="orow")
                ps = pfin.tile([128, NC], F32, tag="pf", name="ps")
                nc.tensor.matmul(ps[0:96, :], s2p[pa], p_sb[0:CD, :],
                                 start=True, stop=True)
                if pa % 2 == 0:
                    nc.vector.tensor_copy(orow, ps[0:96, :])
                else:
                    nc.scalar.copy(orow, ps[0:96, :])
                eng = nc.sync if pa % 2 == 0 else nc.scalar
                eng.dma_start(out=d_out[96 * pa:96 * (pa + 1), :],
                              in_=orow)

    _split_multiwaits(nc)
    _CACHE["nc"] = nc
    return nc


def _make_in_maps(a):
    bf = ml_dtypes.bfloat16
    ei1 = a["edge_index1"].astype(np.int64)
    ei2 = a["edge_index2"].astype(np.int64)
    heads2, tails2 = ei2[0], ei2[1]
    bnbe = 2.0 * np.concatenate([
        a["bn"].reshape(KC, 128).T, a["be"].reshape(KC, 128).T,
    ], axis=1).astype(np.float32)  # [128, 16], col k = 2*(bn||be) chunk k
    # compact output columns: diag (i1*49) first, then other edge cols
    ecols = ei1[0] * N + ei1[1]
    diag = np.arange(N) * (N + 1)
    cc = np.concatenate([diag, np.setdiff1d(np.unique(ecols), diag)])
    assert len(cc) <= NC, f"{len(cc)} compact cols > {NC}"
    colpos = {c: i for i, c in enumerate(cc)}
    b1 = np.zeros((E, NC), bf)
    b1[np.arange(E), [colpos[c] for c in ecols]] = 1
    # rhs [x2^T | ef2^T] pre-permuted to [128, KC*RW] (p-major chunks)
    rhs_f = np.concatenate([a["x2"].T, a["ef2"].T], axis=1)  # [D, RW]
    rhs = np.ascontiguousarray(
        rhs_f.reshape(KC, 128, RW).transpose(1, 0, 2).reshape(128, KC * RW)
    ).astype(bf)
    gw = np.ascontiguousarray(
        a["global_weight"].reshape(KC, 128).T).astype(bf)

    def wtile(W):
        # W^T [din, dout] -> [128, 4*2048]: chunk g = din rows
        # [256g, 256g+256) as two 1024-wide k-slices
        wt = W.T.reshape(4, 2, 128, D).transpose(2, 0, 1, 3)
        return np.ascontiguousarray(wt.reshape(128, 4 * 2 * D)).astype(bf)

    wn = wtile(a["Wn"])
    we = wtile(a["We"])

    in_maps = []
    for c in range(N_CORES):
        owned = np.nonzero(heads2 // I2P == c)[0]
        assert len(owned) <= C, f"core {c} owns {len(owned)} > {C} edges"
        # lhs = [ef1_owned | x1_owned]^T, bf16, pre-permuted [128, KC*LW]
        ef1o = np.zeros((C, D), np.float32)
        ef1o[:len(owned)] = a["ef1"][owned]
        lhs_f = np.concatenate(
            [ef1o.T, a["x1"][I2P * c:I2P * (c + 1)].T], axis=1)  # [D, LW]
        lhs = np.ascontiguousarray(
            lhs_f.reshape(KC, 128, LW).transpose(1, 0, 2)
            .reshape(128, KC * LW)).astype(bf)
        # cv[s, i2] = rotated tail + 64*(i2%2) if head matches else 999;
        # rows C..C+5: route mp-diag delta row C+i2 to output row 64*(i2%2)
        cvm = np.full((CD, 6), 999.0, np.float32)
        for s, j2 in enumerate(owned):
            hl = heads2[j2] - I2P * c
            cvm[s, hl] = (tails2[j2] - I2P * c - hl) % N + 48 * (hl % 2)
        for i2 in range(I2P):
            cvm[C + i2, i2] = 48 * (i2 % 2)
        in_maps.append({
            "Wn": wn, "We": we, "gw": np.ascontiguousarray(gw),
            "bnbe": np.ascontiguousarray(bnbe),
            "lhs": lhs, "rhs": rhs, "b1": b1, "cv": cvm,
        })
    return in_maps


def kernel(**inputs) -> np.ndarray:
    global LAST_RESULTS
    nc = _build()
    a = {k: np.ascontiguousarray(np.asarray(v)) for k, v in inputs.items()}
    in_maps = _make_in_maps(a)
    res = run_bass_kernel_spmd(nc, in_maps, core_ids=list(range(N_CORES)))
    LAST_RESULTS = res

    ei1 = a["edge_index1"].astype(np.int64)
    ecols = ei1[0] * N + ei1[1]
    diag = np.arange(N) * (N + 1)
    cc = np.concatenate([diag, np.setdiff1d(np.unique(ecols), diag)])
    parts = []
    for c in range(N_CORES):
        # scatter compact cols into the (mostly zero) full width, then
        # device rows are [i2l, k2rot, (i1, k1)] with
        # k2g = (k2rot + i2l + 6c) mod 48; want [i2l, i1, (k2g, k1)]
        full = np.zeros((ROWS, COLS), np.float32)
        full[:, cc] = res.results[c]["out"][:, :len(cc)]
        o = full.reshape(I2P, N, N, N).transpose(0, 2, 1, 3)
        o = np.stack([np.roll(o[i], i + I2P * c, axis=1)
                      for i in range(I2P)])
        parts.append(o.reshape(ROWS, COLS))
    return np.concatenate(parts, axis=0).astype(np.float32)


if __name__ == "__main__":
    _build()
    print("build OK")


# revision 69
# speedup vs baseline: 1.0049x; 1.0049x over previous
"""Trainium2 Bass kernel for AffinityMatrixConstructLayer.

Math: M[(i2,i1),(k2,k1)] = sum_{j2,j1} G2[i2,j2]H2[k2,j2] Me[j2,j1]
                            G1[i1,j1]H1[k1,j1]  + diag(Mp)
where Me rows play the j2 role (e1==e2==192 makes the kron index
arithmetic alias me's ef1-row index to j2).

Structure exploited per core c (owns i2 block-rows [6c, 6c+6)):
  - only graph-2 edges with head in range contribute; host permutes
    edges + ef1 rows so the owned slice is compact (C=32 = data max)
  - the output is ~0.4% dense: only graph-1 edge columns + the
    diagonal are nonzero, so the device computes a compact
    [288, NC=256] result (cols = diag48 | edge cols) and the host
    scatters it into the zero matrix
  - bf16 W stream (4MB/core) via SWDGE (nc.gpsimd, 16-engine spray,
    8x512KB tiles all in flight); Wn first so its matvec gather+tanh
    overlaps the We half of the stream
  - coeff = tanh(W@gw+b): PE matvec with gw stationary (1-col
    LDWEIGHTS, host-transposed W), [1,512] psum rows gathered into
    [128,16] via 16 PE transposes; tanh via exp (one ACT table set)
  - one merged affinity GEMM [ef1_own | x1_own] x [x2 | ef2]
  - P = Me_own @ B1c (compact one-hot pattern host-built, bf16)
  - diag(Mp) folded into the final GEMM: p_sb rows C..C+5 carry mp
    rows at compact cols 0:48, cv rows C..C+5 make s2 route them to
    the k2rot=0 output rows. No DRAM bounce, no separate diag add.
  - PE pre-warmed with junk matmuls on the first W tile.
"""

import sys

for _p in ("/opt/trn_rl_repo", "/root/.axon_site/_ro/trn_rl_repo"):
    if _p not in sys.path:
        sys.path.insert(0, _p)

import numpy as np
import ml_dtypes

import concourse.bass as bass
import concourse.mybir as mybir
from concourse.tile import TileContext
from concourse.masks import make_identity
from concourse.bass_utils import run_bass_kernel_spmd

F32 = mybir.dt.float32
BF16 = mybir.dt.bfloat16
AF = mybir.ActivationFunctionType
ALU = mybir.AluOpType

N_CORES = 8
N = 48          # nodes per graph
E = 192         # edges per graph
D = 1024        # feature dim
I2P = N // N_CORES          # 6 block-rows per core
ROWS = I2P * N              # 288 output rows per core
COLS = N * N                # 2304
C = 32                      # padded owned-edge capacity per core
CD = C + 6                  # + 6 mp-diagonal delta rows
LW = C + 6                  # merged affinity lhs width (ef1_own | x1_own)
RW = N + E                  # merged affinity rhs width (x2 | ef2) = 240
KC = D // 128               # 8 contraction chunks
NC = 256                    # padded compact output columns (diag48 | edge cols)

_CACHE: dict = {}
LAST_RESULTS = None


def _split_multiwaits(nc):
    """This walrus build encodes at most one sync-wait per instruction.
    Move extra waits onto injected single-wait drains on the same engine
    (engine queues execute in order, so semantics are preserved)."""
    for f in nc.m.functions:
        for blk in f.blocks:
            out = []
            for inst in blk.instructions:
                si = getattr(inst, "sync_info", None)
                if si is not None and si.on_wait and len(si.on_wait) > 1:
                    waits = list(si.on_wait)
                    for w in waits[:-1]:
                        d = mybir.InstDrain(
                            name=nc.get_next_instruction_name(),
                            ins=[], outs=[], bass_is_fusable=False)
                        d.engine = inst.engine
                        d.sync_info = mybir.SyncInfo(on_wait=[w], on_update=[])
                        out.append(d)
                    si.on_wait = waits[-1:]
                out.append(inst)
            try:
                blk.instructions[:] = out
            except TypeError:
                blk.instructions = out


def _build() -> bass.Bass:
    if "nc" in _CACHE:
        return _CACHE["nc"]
    nc = bass.Bass(trn_type="TRN2", num_devices=N_CORES)

    # Wn/We are host-TRANSPOSED+tiled: [128, 4*2048]; column-chunk g holds
    # din rows [256g, 256g+256) as two 1024-wide k-slices, so the PE
    # matvec can use gw as the stationary operand (1-col LDWEIGHTS)
    d_Wn = nc.dram_tensor("Wn", [128, KC * D], BF16,
                          kind="ExternalInput")
    d_We = nc.dram_tensor("We", [128, KC * D], BF16,
                          kind="ExternalInput")
    d_gw = nc.dram_tensor("gw", [128, KC], BF16, kind="ExternalInput")
    d_bnbe = nc.dram_tensor("bnbe", [128, 16], F32, kind="ExternalInput")
    d_lhs = nc.dram_tensor("lhs", [128, KC * LW], BF16, kind="ExternalInput")
    d_rhs = nc.dram_tensor("rhs", [128, KC * RW], BF16, kind="ExternalInput")
    d_b1 = nc.dram_tensor("b1", [E, NC], BF16, kind="ExternalInput")
    d_cv = nc.dram_tensor("cv", [CD, 6], F32, kind="ExternalInput")
    d_out = nc.dram_tensor("out", [ROWS, NC], F32, kind="ExternalOutput")

    with TileContext(nc) as tc:
        with (
            tc.tile_pool(name="const", bufs=1) as cpool,
            tc.tile_pool(name="wstream", bufs=8) as wpool,
            tc.tile_pool(name="scratch", bufs=2) as spool,
            tc.tile_pool(name="orow", bufs=3) as opool,
            tc.tile_pool(name="paff", bufs=1, space="PSUM") as paff,
            tc.tile_pool(name="pmv", bufs=1, space="PSUM") as pmv,
            tc.tile_pool(name="pout", bufs=2, space="PSUM") as pout,
            tc.tile_pool(name="pfin", bufs=2, space="PSUM") as pfin,
        ):
            # ---- bulk stream on SWDGE (gpsimd): 16-engine spray, 512KB
            # chunks. Emitted FIRST so Q7 descriptor-gen precedes its
            # other (iota/memset) work.
            wn_tiles, we_tiles = [], []
            for g in range(4):
                wn = wpool.tile([128, 2 * D], BF16, tag="w", name=f"wn{g}")
                nc.gpsimd.dma_start(
                    out=wn, in_=d_Wn[:, 2 * D * g:2 * D * (g + 1)])
                wn_tiles.append(wn)
            for g in range(4):
                we = wpool.tile([128, 2 * D], BF16, tag="w", name=f"we{g}")
                nc.gpsimd.dma_start(
                    out=we, in_=d_We[:, 2 * D * g:2 * D * (g + 1)])
                we_tiles.append(we)
            # ---- everything else on the two HWDGE queues (idle during
            # the SWDGE W stream): sync = bb, rhs, lhs, b1_hi;
            # scalar = cv, gw, b1_lo
            bb_t = cpool.tile([128, 16], F32, tag="bb", name="bb")
            nc.sync.dma_start(out=bb_t, in_=d_bnbe[:, :])
            rhs = cpool.tile([128, KC * RW], BF16, tag="rhs", name="rhs")
            nc.sync.dma_start(out=rhs, in_=d_rhs[:, :])
            lhs = cpool.tile([128, KC * LW], BF16, tag="lhs", name="lhs")
            nc.sync.dma_start(out=lhs, in_=d_lhs[:, :])
            b1_hi = cpool.tile([128, NC], BF16, tag="b1_hi", name="b1_hi")
            nc.sync.dma_start(out=b1_hi, in_=d_b1[0:128, :])
            cv = cpool.tile([CD, 6], F32, tag="cv", name="cv")
            nc.scalar.dma_start(out=cv, in_=d_cv[:, :])
            gwp = cpool.tile([128, KC], BF16, tag="gwp", name="gwp")
            nc.scalar.dma_start(out=gwp, in_=d_gw[:, :])
            b1_lo = cpool.tile([64, NC], BF16, tag="b1_lo", name="b1_lo")
            nc.scalar.dma_start(out=b1_lo, in_=d_b1[128:192, :])

            # ---------- constants (built during stream) -----------------
            ident = cpool.tile([128, 128], F32, tag="ident", name="ident")
            make_identity(nc, ident)
            iota96 = cpool.tile([CD, 96], F32, tag="iota96", name="i96")
            nc.gpsimd.iota(iota96, pattern=[[1, 96]], base=0,
                           channel_multiplier=0,
                           allow_small_or_imprecise_dtypes=True)

            # ACT table preload overlaps the W stream
            dum = spool.tile([1, 1], F32, tag="dum", name="dum")
            nc.vector.memset(dum, 0.0)
            nc.scalar.activation(dum, dum, AF.Exp)

            # p_sb rows C..C+5 (mp diag deltas): zero the background once
            # (tile padded to 64 partitions for gpsimd's 32-alignment rule)
            p_sb = cpool.tile([64, NC], BF16, tag="p_sb", name="p_sb")
            nc.gpsimd.memset(p_sb, 0.0)

            # s2 per pair: col (64*(i2%2) + k2rot) hot iff cv matches;
            # rows C..C+5 route the mp-diag delta rows of p_sb
            s2p = []
            for pa in range(3):
                sa = spool.tile([CD, 96], F32, tag="s2a", name=f"s2a{pa}")
                nc.vector.tensor_tensor(
                    sa, iota96,
                    cv[:, 2 * pa:2 * pa + 1].broadcast_to((CD, 96)),
                    ALU.is_equal)
                sb = spool.tile([CD, 96], F32, tag="s2b", name=f"s2b{pa}")
                nc.vector.tensor_tensor(
                    sb, iota96,
                    cv[:, 2 * pa + 1:2 * pa + 2].broadcast_to((CD, 96)),
                    ALU.is_equal)
                st = cpool.tile([CD, 96], BF16, tag=f"s2{pa}", name=f"s2{pa}")
                nc.vector.tensor_tensor(st, sa, sb, ALU.add)
                s2p.append(st)

            # ---------- streaming PE matvec: gw stationary (1-col weights)
            # Wn fully streamed first, so its gather+tanh overlaps the We
            # stream. Junk matmuls on the first tile warm the HAM early.
            ones33 = cpool.tile([33, 1], F32, tag="ones33", name="ones33")
            nc.vector.memset(ones33, 1.0)
            coeff = cpool.tile([128, 16], F32, tag="coeff", name="coeff")
            pmvt = pmv.tile([128, 16], F32, tag="pmvt", name="pmvt")

            warm = pfin.tile([128, 512], F32, tag="pf", name="wrm")
            for j in range(6):
                nc.tensor.matmul(warm, wn_tiles[0][:, 0:128],
                                 wn_tiles[0][:, 0:512],
                                 start=True, stop=True)

            def matvec_half(m, tiles, pm):
                ntile = len(tiles)
                spt = KC // ntile  # k-slices per tile
                for g in range(ntile):
                    for s in range(spt):
                        k = spt * g + s
                        for h in range(2):
                            nc.tensor.matmul(
                                pm[32 * h:32 * h + 1, :], gwp[:, k:k + 1],
                                tiles[g][:, 1024 * s + 512 * h:
                                         1024 * s + 512 * h + 512],
                                start=(k == 0), stop=(k == KC - 1))
                # gather [1,512] psum rows -> pmvt cols via PE transposes
                mvs = spool.tile([33, 512], F32, tag=f"mvs{m}", name=f"mvs{m}")
                nc.vector.tensor_copy(mvs[:, 0:256], pm[:, 0:256])
                nc.scalar.copy(mvs[:, 256:512], pm[:, 256:512])
                for h in range(2):
                    for kc in range(4):
                        col = m * 8 + 4 * h + kc
                        nc.tensor.transpose(
                            pmvt[:, col:col + 1],
                            mvs[32 * h:32 * h + 1, 128 * kc:128 * kc + 128],
                            ones33[32 * h:32 * h + 1, 0:1])
                # tanh(mv + b) via exp on this half's 8 columns
                cs = slice(m * 8, m * 8 + 8)
                mv2 = spool.tile([128, 8], F32, tag=f"mv2{m}", name=f"mv2{m}")
                nc.vector.scalar_tensor_tensor(
                    out=mv2, in0=pmvt[:, cs], scalar=2.0, op0=ALU.mult,
                    in1=bb_t[:, cs], op1=ALU.add)
                et = spool.tile([128, 8], F32, tag=f"et{m}", name=f"et{m}")
                nc.scalar.activation(et, mv2, AF.Exp)
                nc.vector.tensor_scalar_add(et, et, 1.0)
                rt = spool.tile([128, 8], F32, tag=f"rt{m}", name=f"rt{m}")
                nc.vector.reciprocal(rt, et)
                nc.vector.tensor_scalar(coeff[:, cs], rt, -2.0, 1.0,
                                        ALU.mult, ALU.add)

            pmva = pmv.tile([33, 512], F32, tag="pmva", name="pmva")
            matvec_half(0, wn_tiles, pmva)
            pmvb = pmv.tile([33, 512], F32, tag="pmvb", name="pmvb")
            matvec_half(1, we_tiles, pmvb)

            # ---------- scaled lhs + merged affinity GEMM ---------------
            al = cpool.tile([128, KC * LW], BF16, tag="al", name="al")
            al3 = al.rearrange("p (k n) -> p k n", n=LW)
            lhs3 = lhs.rearrange("p (k n) -> p k n", n=LW)
            nc.vector.tensor_tensor(
                al3[:, :, 0:C], lhs3[:, :, 0:C],
                coeff[:, KC:16].unsqueeze(2).broadcast_to((128, KC, C)),
                ALU.mult)
            nc.vector.tensor_tensor(
                al3[:, :, C:LW], lhs3[:, :, C:LW],
                coeff[:, 0:KC].unsqueeze(2).broadcast_to((128, KC, I2P)),
                ALU.mult)
            aff = paff.tile([LW, RW], F32, tag="aff", name="aff")
            rhs3 = rhs.rearrange("p (k n) -> p k n", n=RW)
            for k in range(KC):
                nc.tensor.matmul(aff, al3[:, k, :], rhs3[:, k, :],
                                 start=(k == 0), stop=(k == KC - 1))


            # ---------- softplus_relu: relu(relu(x) + ln(e^-.5 + e^(-|x|-.5)))
            nh05 = cpool.tile([LW, 1], F32, tag="nh05", name="nh05")
            nc.vector.memset(nh05, -0.5)
            lnc = cpool.tile([LW, 1], F32, tag="lnc", name="lnc")
            nc.vector.memset(lnc, 0.6065306597126334)
            ab = spool.tile([LW, RW], F32, tag="sp_ab", name="sp_ab")
            nc.scalar.activation(ab, aff, AF.Abs)
            ex = spool.tile([LW, RW], F32, tag="sp_ex", name="sp_ex")
            nc.scalar.activation(ex, ab, AF.Exp, scale=-1.0, bias=nh05)
            ln = spool.tile([LW, RW], F32, tag="sp_ln", name="sp_ln")
            nc.scalar.activation(ln, ex, AF.Ln, bias=lnc)
            pre = spool.tile([LW, RW], F32, tag="sp_pre", name="sp_pre")
            nc.vector.scalar_tensor_tensor(
                out=pre, in0=aff, scalar=0.0, op0=ALU.max,
                in1=ln, op1=ALU.add)
            # (final relu is folded into the consumers below)

            # mp diag deltas: compact cols 0..47 ARE the diagonal, so this
            # is a plain relu-copy into p_sb delta rows (partitions C..C+5)
            nc.vector.scalar_tensor_tensor(
                out=p_sb[C:CD, 0:N], in0=pre[C:CD, 0:N], scalar=0.0,
                op0=ALU.max, in1=pre[C:CD, 0:N], op1=ALU.bypass)

            # MeT via PE transposes (relu folded into the psum->sbuf copy)
            ptm1 = pout.tile([128, C], F32, tag="po", name="ptm1")
            nc.tensor.transpose(ptm1, pre[0:C, N:N + 128], ident[0:C, 0:C])
            met_hi = cpool.tile([128, C], BF16, tag="met_hi", name="met_hi")
            nc.scalar.activation(met_hi, ptm1, AF.Relu)
            ptm2 = pout.tile([64, C], F32, tag="po", name="ptm2")
            nc.tensor.transpose(ptm2, pre[0:C, N + 128:N + 192],
                                ident[0:C, 0:C])
            met_lo = cpool.tile([64, C], BF16, tag="met_lo", name="met_lo")
            nc.scalar.activation(met_lo, ptm2, AF.Relu)

            # ---------- P = Me_own @ B1c  [C, 256] -> bf16 p_sb ---------
            pp = pout.tile([C, NC], F32, tag="po", name="pp")
            nc.tensor.matmul(pp, met_hi, b1_hi, start=True, stop=False)
            nc.tensor.matmul(pp, met_lo, b1_lo, start=False, stop=True)
            nc.vector.tensor_copy(p_sb[0:C, 0:NC // 2], pp[:, 0:NC // 2])
            nc.scalar.copy(p_sb[0:C, NC // 2:], pp[:, NC // 2:])

            # ---------- finals: orow = s2^T @ p_sb per pair + out DMA ---
            for pa in range(3):
                i2a, i2b = 2 * pa, 2 * pa + 1
                orow = opool.tile([96, NC], F32, tag="orow", name="orow")
                ps = pfin.tile([128, NC], F32, tag="pf", name="ps")
                nc.tensor.matmul(ps[0:96, :], s2p[pa], p_sb[0:CD, :],
                                 start=True, stop=True)
                if pa % 2 == 0:
                    nc.vector.tensor_copy(orow, ps[0:96, :])
                else:
                    nc.scalar.copy(orow, ps[0:96, :])
                eng = nc.sync if pa % 2 == 0 else nc.scalar
                eng.dma_start(out=d_out[96 * pa:96 * (pa + 1), :],
                              in_=orow)

    _split_multiwaits(nc)
    _CACHE["nc"] = nc
    return nc


def _make_in_maps(a):
    bf = ml_dtypes.bfloat16
    ei1 = a["edge_index1"].astype(np.int64)
    ei2 = a["edge_index2"].astype(np.int64)
    heads2, tails2 = ei2[0], ei2[1]
    bnbe = 2.0 * np.concatenate([
        a["bn"].reshape(KC, 128).T, a["be"].reshape(KC, 128).T,
    ], axis=1).astype(np.float32)  # [128, 16], col k = 2*(bn||be) chunk k
    # compact output columns: diag (i1*49) first, then other edge cols
    ecols = ei1[0] * N + ei1[1]
    diag = np.arange(N) * (N + 1)
    cc = np.concatenate([diag, np.setdiff1d(np.unique(ecols), diag)])
    assert len(cc) <= NC, f"{len(cc)} compact cols > {NC}"
    colpos = {c: i for i, c in enumerate(cc)}
    b1 = np.zeros((E, NC), bf)
    b1[np.arange(E), [colpos[c] for c in ecols]] = 1
    # rhs [x2^T | ef2^T] pre-permuted to [128, KC*RW] (p-major chunks)
    rhs_f = np.concatenate([a["x2"].T, a["ef2"].T], axis=1)  # [D, RW]
    rhs = np.ascontiguousarray(
        rhs_f.reshape(KC, 128, RW).transpose(1, 0, 2).reshape(128, KC * RW)
    ).astype(bf)
    gw = np.ascontiguousarray(
        a["global_weight"].reshape(KC, 128).T).astype(bf)

    def wtile(W):
        # W^T [din, dout] -> [128, 4*2048]: chunk g = din rows
        # [256g, 256g+256) as two 1024-wide k-slices
        wt = W.T.reshape(4, 2, 128, D).transpose(2, 0, 1, 3)
        return np.ascontiguousarray(wt.reshape(128, 4 * 2 * D)).astype(bf)

    wn = wtile(a["Wn"])
    we = wtile(a["We"])

    in_maps = []
    for c in range(N_CORES):
        owned = np.nonzero(heads2 // I2P == c)[0]
        assert len(owned) <= C, f"core {c} owns {len(owned)} > {C} edges"
        # lhs = [ef1_owned | x1_owned]^T, bf16, pre-permuted [128, KC*LW]
        ef1o = np.zeros((C, D), np.float32)
        ef1o[:len(owned)] = a["ef1"][owned]
        lhs_f = np.concatenate(
            [ef1o.T, a["x1"][I2P * c:I2P * (c + 1)].T], axis=1)  # [D, LW]
        lhs = np.ascontiguousarray(
            lhs_f.reshape(KC, 128, LW).transpose(1, 0, 2)
            .reshape(128, KC * LW)).astype(bf)
        # cv[s, i2] = rotated tail + 64*(i2%2) if head matches else 999;
        # rows C..C+5: route mp-diag delta row C+i2 to output row 64*(i2%2)
        cvm = np.full((CD, 6), 999.0, np.float32)
        for s, j2 in enumerate(owned):
            hl = heads2[j2] - I2P * c
            cvm[s, hl] = (tails2[j2] - I2P * c - hl) % N + 48 * (hl % 2)
        for i2 in range(I2P):
            cvm[C + i2, i2] = 48 * (i2 % 2)
        in_maps.append({
            "Wn": wn, "We": we, "gw": np.ascontiguousarray(gw),
            "bnbe": np.ascontiguousarray(bnbe),
            "lhs": lhs, "rhs": rhs, "b1": b1, "cv": cvm,
        })
    return in_maps


def kernel(**inputs) -> np.ndarray:
    global LAST_RESULTS
    nc = _build()
    a = {k: np.ascontiguousarray(np.asarray(v)) for k, v in inputs.items()}
    in_maps = _make_in_maps(a)
    res = run_bass_kernel_spmd(nc, in_maps, core_ids=list(range(N_CORES)))
    LAST_RESULTS = res

    ei1 = a["edge_index1"].astype(np.int64)
    ecols = ei1[0] * N + ei1[1]
    diag = np.arange(N) * (N + 1)
    cc = np.concatenate([diag, np.setdiff1d(np.unique(ecols), diag)])
    parts = []
    for c in range(N_CORES):
        # scatter compact cols into the (mostly zero) full width, then
        # device rows are [i2l, k2rot, (i1, k1)] with
        # k2g = (k2rot + i2l + 6c) mod 48; want [i2l, i1, (k2g, k1)]
        full = np.zeros((ROWS, COLS), np.float32)
        full[:, cc] = res.results[c]["out"][:, :len(cc)]
        o = full.reshape(I2P, N, N, N).transpose(0, 2, 1, 3)
        o = np.stack([np.roll(o[i], i + I2P * c, axis=1)
                      for i in range(I2P)])
        parts.append(o.reshape(ROWS, COLS))
    return np.concatenate(parts, axis=0).astype(np.float32)


if __name__ == "__main__":
    _build()
    print("build OK")
